# revision 8
# baseline (speedup 1.0000x reference)
"""Trainium2 Bass kernel for the ArcModel3Phase loss.

Math restructuring (vs the reference):
  Each MC interface term needs lse_m = logsumexp_n(lpx + lpy + lptx) over
  N=1024 samples for each of M points.  Expanding all three log-densities,
      l_nm = A_m + B_n + x_m*(tx_n/sn^2) + y_m*(2 G_n/sn^2)
             + log(1 - exp(-(4/sn^2) y_m G_n))
  The affine part R1_nm = x_m*txp_n + y_m*g1_n + Bp_n is a K=3 matmul
  (TensorEngine), and with w_nm = (4/sn^2) y G = R1 - R2 where
  R2_nm = x_m*txp_n - y_m*g1_n + Bp_n (same matmul, g1 row negated),
      sum_n e^{l - max} = sum_n e^{R1-max} - sum_n e^{R2-max}
  (A_m cancels in the stabilized sums; added back at the end).
  w >= 0.21 for this model, so the subtraction loses at most ~2.4 bits.
  Per-n rows (txp, g1, Bp) are O(N) host-precomputed constants.

Per-core layout: M=100000 sharded 8 ways -> 12500, padded to 12544 =
128 partitions x 98 tiles (m = p*98 + t), with a 0/1 mask for the pad.
Hot loop per tile per term: 4 fp32 matmuls (2x512 per region) -> PSUM,
one DVE negated max-reduce of R1, two ScalarE Exp passes with fused
accumulate (softmax pattern).  ScalarE is the predicted bottleneck at
~6 x 1024 cycles per tile.
"""
import math

import numpy as np
from scipy.special import erf, erfinv

import concourse.bass as bass
import concourse.tile as tile
from concourse import bacc, mybir
from concourse import bass_isa
from concourse import masks
from concourse.bass_utils import run_bass_kernel_spmd

WF = 3.0
LOG2PI = math.log(2.0 * math.pi)
M = 100_000
N_MC = 1024
N_CORES = 8
M_CORE = M // N_CORES          # 12500
P = 128
T = 98                         # tiles per core; P*T = 12544 >= M_CORE
M_PAD = P * T

_graph_cache = {}


def _host_rows(ku, Ia, Ib, sigma_b, sigma_n, logw):
    """Host-precomputed per-n rows for one interface term (float64 math)."""
    ku = ku.astype(np.float64)
    sn2 = sigma_n ** 2
    I_min = Ia + 0.5 * (Ib - Ia) * (1.0 + erf(-WF / np.sqrt(2.0)))
    I_diff = (Ib - Ia) * erf(WF / np.sqrt(2.0))
    tx = ku * I_diff + I_min
    ei = erfinv(2.0 * (tx - Ia) / (Ib - Ia) - 1.0)
    G = (Ib - Ia) / np.sqrt(2.0 * np.pi * sigma_b ** 2) * np.exp(-ei ** 2)
    lptx = -np.log(2.0 * WF * (Ib - Ia)) + 0.5 * LOG2PI + ei ** 2
    B = -0.5 * tx ** 2 / sn2 - np.log(G) - G ** 2 / sn2 + lptx
    C0 = (-np.log(sigma_n) - 0.5 * LOG2PI
          + np.log(2.0) - 2.0 * np.log(sigma_n)
          + 0.5 * np.log(2.0 / np.pi) - np.log(2.0)
          - 0.5 * np.log(2.0) + np.log(sigma_n))
    Bp = B + np.log(I_diff) - np.log(N_MC) + logw + C0
    return (tx / sn2).astype(np.float32), (2.0 * G / sn2).astype(np.float32), \
        Bp.astype(np.float32)


def _build_bass(sigma_n, I1, I2, I3, logw):
    """Builds the SPMD kernel graph. Scalars are compile-time constants."""
    nc = bacc.Bacc("TRN2", target_bir_lowering=False, debug=False,
                   num_devices=N_CORES)
    dt = mybir.dt.float32
    f = mybir.ActivationFunctionType
    alu = mybir.AluOpType

    x_d = nc.dram_tensor("x", [M_PAD], dt, kind="ExternalInput").ap()
    y_d = nc.dram_tensor("y", [M_PAD], dt, kind="ExternalInput").ap()
    mask_d = nc.dram_tensor("mask", [M_PAD], dt, kind="ExternalInput").ap()
    # rhs[j, r, k, n]: term j, region r (0: +g1, 1: -g1), row k in (txp, g1, Bp)
    rhs_d = nc.dram_tensor("rhs", [3, 2, 3, N_MC], dt, kind="ExternalInput").ap()
    out_d = nc.dram_tensor("out", [1], dt, kind="ExternalOutput").ap()

    sn = sigma_n
    ck = (math.log(2.0) - math.lgamma(1.5) - 4.0 * math.log(sn)
          - 0.5 * LOG2PI)

    with tile.TileContext(nc) as tc:
        with (
            tc.tile_pool(name="singles", bufs=1) as singles,
            tc.tile_pool(name="work", bufs=2) as work,
            tc.tile_pool(name="psumA", bufs=2, space="PSUM") as psumA,
            tc.tile_pool(name="psumB", bufs=2, space="PSUM") as psumB,
            tc.tile_pool(name="dump", bufs=3) as dump,
        ):
            # ---- load inputs ----
            xs = singles.tile([P, T], dt, tag="xs")
            ys = singles.tile([P, T], dt, tag="ys")
            msk = singles.tile([P, T], dt, tag="msk")
            nc.sync.dma_start(xs[:], x_d.rearrange("(p t) -> p t", p=P))
            nc.sync.dma_start(ys[:], y_d.rearrange("(p t) -> p t", p=P))
            nc.sync.dma_start(msk[:], mask_d.rearrange("(p t) -> p t", p=P))
            # SBUF rows layout: [k, j, r, n] so lhsT row k is the partition
            rhs = singles.tile([3, 3, 2, N_MC], dt, tag="rhs")
            nc.sync.dma_start(rhs[:], rhs_d.rearrange("j r k n -> k j r n"))

            ident = singles.tile([P, P], dt, tag="ident")
            masks.make_identity(nc, ident[:])

            # ---- per-m planes ----
            # sx2h = x^2/(2 sn^2); y2s = (y/sn)^2; lny = ln y
            sx2h = singles.tile([P, T], dt, tag="sx2h")
            y2s = singles.tile([P, T], dt, tag="y2s")
            lny = singles.tile([P, T], dt, tag="lny")
            nc.scalar.activation(sx2h[:], xs[:], f.Square,
                                 scale=1.0 / (sn * math.sqrt(2.0)))
            nc.scalar.activation(y2s[:], ys[:], f.Square, scale=1.0 / sn)
            nc.scalar.activation(lny[:], ys[:], f.Ln)
            # A = lny - sx2h - y2s
            A = singles.tile([P, T], dt, tag="A")
            tmpA = work.tile([P, T], dt, tag="tmpA")
            nc.vector.scalar_tensor_tensor(tmpA[:], sx2h[:], 1.0, y2s[:],
                                           alu.mult, alu.add)
            nc.vector.scalar_tensor_tensor(A[:], tmpA[:], -1.0, lny[:],
                                           alu.mult, alu.add)

            # Stationary operand for all tiles: LT_ALL[k, t, p] with rows
            # k = (x, y, mask); mask doubles as the "ones" row (pad entries
            # are masked out of the final sum anyway).  lhsT for tile t is
            # LT_ALL[:, t, :] — base partition 0 as the PE requires.
            LT_ALL = singles.tile([3, T, P], dt, tag="LT_ALL")
            for k, src in enumerate((xs, ys, msk)):
                tp = psumA.tile([T, P], dt, tag="ra", name=f"tp{k}")
                nc.tensor.transpose(tp[:], src[:], ident[:])
                st = work.tile([T, P], dt, tag="st", name=f"st{k}")
                nc.vector.tensor_copy(st[:], tp[:])
                nc.sync.dma_start(LT_ALL[k: k + 1, :, :], st[:])

            # interior planes -> PL[:, :, 0..2]
            PL = singles.tile([P, T, 6], dt, tag="PL")
            base = singles.tile([P, T], dt, tag="base")
            nc.vector.scalar_tensor_tensor(base[:], lny[:], 2.0, y2s[:],
                                           alu.mult, alu.subtract)
            for k, I in enumerate((I1, I2, I3)):
                qb = work.tile([P, 1], dt, tag="qb")
                nc.vector.memset(qb[:], -I / sn)
                q = work.tile([P, T], dt, tag="q")
                nc.scalar.activation(q[:], xs[:], f.Square,
                                     scale=1.0 / sn, bias=qb[:])
                basek = work.tile([P, T], dt, tag="basek")
                nc.vector.tensor_scalar_add(basek[:], base[:],
                                            ck + float(logw[k]))
                nc.vector.scalar_tensor_tensor(PL[:, :, k], q[:], -0.5,
                                               basek[:], alu.mult, alu.add)

            # ---- hot loop ----
            NM = [singles.tile([P, T], dt, tag=f"NM{j}", name=f"NM{j}")
                  for j in range(3)]
            S1 = [singles.tile([P, T], dt, tag=f"S1{j}", name=f"S1{j}")
                  for j in range(3)]
            S2 = [singles.tile([P, T], dt, tag=f"S2{j}", name=f"S2{j}")
                  for j in range(3)]
            for t in range(T):
                lhsT = LT_ALL[:, t, :]
                for j in range(3):
                    r1 = psumA.tile([P, N_MC], dt, tag="ra")
                    r2 = psumB.tile([P, N_MC], dt, tag="rb")
                    for h in range(2):
                        sl = slice(h * 512, h * 512 + 512)
                        nc.tensor.matmul(r1[:, sl], lhsT, rhs[:, j, 0, sl],
                                         start=True, stop=True)
                        nc.tensor.matmul(r2[:, sl], lhsT, rhs[:, j, 1, sl],
                                         start=True, stop=True)
                    nc.vector.tensor_reduce(NM[j][:, t: t + 1], r1[:],
                                            mybir.AxisListType.X, alu.max,
                                            negate=True)
                    e1 = dump.tile([P, N_MC], dt, tag="e")
                    nc.scalar.activation(e1[:], r1[:], f.Exp,
                                         bias=NM[j][:, t: t + 1],
                                         accum_out=S1[j][:, t: t + 1])
                    e2 = dump.tile([P, N_MC], dt, tag="e")
                    nc.scalar.activation(e2[:], r2[:], f.Exp,
                                         bias=NM[j][:, t: t + 1],
                                         accum_out=S2[j][:, t: t + 1])

            # ---- finalize interface planes: PL[3+j] = A - NM + ln(S1-S2) ----
            for j in range(3):
                sd = work.tile([P, T], dt, tag="sd")
                nc.vector.scalar_tensor_tensor(sd[:], S2[j][:], -1.0, S1[j][:],
                                               alu.mult, alu.add)
                lnsd = work.tile([P, T], dt, tag="lnsd")
                nc.scalar.activation(lnsd[:], sd[:], f.Ln)
                v = work.tile([P, T], dt, tag="v")
                nc.vector.scalar_tensor_tensor(v[:], NM[j][:], -1.0,
                                               lnsd[:], alu.mult, alu.add)
                nc.vector.scalar_tensor_tensor(PL[:, :, 3 + j], v[:], 1.0,
                                               A[:], alu.mult, alu.add)

            # ---- final mix: logsumexp over the 6 planes ----
            mx6 = singles.tile([P, T, 1], dt, tag="mx6")
            nc.vector.tensor_reduce(mx6[:], PL[:], mybir.AxisListType.X,
                                    alu.max)
            D = singles.tile([P, T, 6], dt, tag="D")
            nc.vector.tensor_tensor(D[:], PL[:],
                                    mx6[:].broadcast_to([P, T, 6]),
                                    alu.subtract)
            E = singles.tile([P, T, 6], dt, tag="E")
            nc.scalar.activation(E[:], D[:], f.Exp)
            sm = singles.tile([P, T, 1], dt, tag="sm")
            nc.vector.tensor_reduce(sm[:], E[:], mybir.AxisListType.X, alu.add)
            lnm = singles.tile([P, T], dt, tag="lnm")
            nc.scalar.activation(lnm[:], sm[:, :, 0], f.Ln)
            logmix = singles.tile([P, T], dt, tag="logmix")
            nc.vector.tensor_tensor(logmix[:], lnm[:], mx6[:, :, 0], alu.add)

            # ---- masked sum over all m; negate on host ----
            colsum = singles.tile([P, 1], dt, tag="colsum")
            dmp = work.tile([P, T], dt, tag="dmp")
            nc.vector.scalar_tensor_tensor(dmp[:], logmix[:], 1.0, msk[:],
                                           alu.mult, alu.mult,
                                           accum_out=colsum[:])
            total = singles.tile([P, 1], dt, tag="total")
            nc.gpsimd.partition_all_reduce(total[:], colsum[:], channels=P,
                                           reduce_op=bass_isa.ReduceOp.add)
            nc.sync.dma_start(out_d.rearrange("(p o) -> p o", p=1),
                              total[0:1, 0:1])

    nc.compile()
    return nc


def kernel(x, y, ku12, ku23, ku13, sigma_b, sigma_n, I1, I2, I3, w):
    x = np.asarray(x, np.float32)
    y = np.asarray(y, np.float32)
    sigma_b = float(sigma_b)
    sigma_n = float(sigma_n)
    I1, I2, I3 = float(I1), float(I2), float(I3)
    w64 = np.asarray(w, np.float64)
    logw = w64 - (np.log(np.sum(np.exp(w64 - w64.max()))) + w64.max())

    rows = np.empty((3, 2, 3, N_MC), np.float32)
    for j, (ku, Ia, Ib) in enumerate(((ku12, I1, I2), (ku23, I2, I3),
                                      (ku13, I1, I3))):
        txp, g1, Bp = _host_rows(np.asarray(ku), Ia, Ib, sigma_b, sigma_n,
                                 float(logw[3 + j]))
        rows[j, 0] = np.stack([txp, g1, Bp])
        rows[j, 1] = np.stack([txp, -g1, Bp])

    mask = np.zeros(M_PAD, np.float32)
    mask[:M_CORE] = 1.0

    key = (sigma_n, I1, I2, I3, tuple(np.round(logw, 12)))
    if key not in _graph_cache:
        _graph_cache[key] = _build_bass(sigma_n, I1, I2, I3, logw)
    nc = _graph_cache[key]

    in_maps = []
    for i in range(N_CORES):
        xi = np.full(M_PAD, 0.5, np.float32)
        yi = np.full(M_PAD, 0.5, np.float32)
        xi[:M_CORE] = x[i * M_CORE: (i + 1) * M_CORE]
        yi[:M_CORE] = y[i * M_CORE: (i + 1) * M_CORE]
        in_maps.append({"x": xi, "y": yi, "mask": mask, "rhs": rows})

    res = run_bass_kernel_spmd(nc, in_maps, core_ids=list(range(N_CORES)))
    global _last_results
    _last_results = res
    partials = [float(res.results[i]["out"][0]) for i in range(N_CORES)]
    return np.float32(-np.sum(partials))


_last_results = None


# revision 13
# speedup vs baseline: 4.5309x; 4.5309x over previous
"""Trainium2 Bass kernel for the ArcModel3Phase loss.

Math restructuring (vs the reference):
  Each MC interface term needs lse_m = logsumexp_n(lpx + lpy + lptx) over
  N=1024 samples for each of M points.  Expanding all three log-densities,
      l_nm = A_m + B_n + x_m*(tx_n/sn^2) + y_m*(2 G_n/sn^2)
             + log(1 - exp(-(4/sn^2) y_m G_n))
  The affine part R1_nm = x_m*txp_n + y_m*g1_n + B_n is a matmul, and with
  w_nm = (4/sn^2) y G = R1 - R2 where R2_nm = x_m*txp_n - y_m*g1_n + B_n
  (same matmul, g1 negated),
      sum_n e^{l - b} = sum_n e^{R1-b} - sum_n e^{R2-b}
  for any bound b (A_m cancels; b only affects numerics and a drop of up
  to (Ib-Ia)^2/(2 sn^2) ~ 72 below the true max still keeps every f32
  intermediate finite).  w >= 0.2 here, so the subtraction loses < 3 bits.

Device mapping:
  - fp32 matmul streams at 1/4 rate on the PE, so each factor is split
    hi/lo into bf16 (x*t = xh*th + xh*tl + xl*th, ~2^-17 relative) and the
    affine part becomes a K=8 bf16 matmul at full rate.
  - b is a negated max over a stride-8 subsample of R1 (exact in the
    result, see above).
  - Samples are sorted by G on the host; those with w >= W_SKIP for every
    m (G >= W_SKIP*sn^2/(4 y_min)) contribute < e^-W_SKIP relative to s2
    and are skipped in the R2/e2 pass entirely.
  - ScalarE does e1/e2 with the fused accumulate (softmax pattern).

Per-core layout: M=100000 sharded 8 ways -> 12500, padded to 12544 =
128 partitions x 98 tiles (m = p*98 + t), with a 0/1 mask for the pad.
The mask is also the "ones" lhsT row (pad entries are masked out at the
end, so their garbage B contribution is irrelevant).
"""
import math

import numpy as np
import ml_dtypes
from scipy.special import erf, erfinv

import concourse.bass as bass
import concourse.tile as tile
from concourse import bacc, mybir
from concourse import bass_isa
from concourse.bass_utils import run_bass_kernel_spmd

WF = 3.0
LOG2PI = math.log(2.0 * math.pi)
M = 100_000
N_MC = 1024
N_CORES = 8
M_CORE = M // N_CORES          # 12500
P = 128
T = 98                         # tiles per core; P*T = 12544 >= M_CORE
M_PAD = P * T
W_SKIP = 9.0                   # skip e2 samples with w >= this for all m
BF16 = ml_dtypes.bfloat16

_graph_cache = {}
_last_results = None


def _split(a):
    hi = a.astype(BF16)
    lo = (a - hi.astype(np.float64)).astype(BF16)
    return hi, lo


def _host_rows(ku, Ia, Ib, sigma_b, sigma_n, logw):
    """Per-n rows for one interface term (float64 math), G-sorted."""
    ku = ku.astype(np.float64)
    sn2 = sigma_n ** 2
    I_min = Ia + 0.5 * (Ib - Ia) * (1.0 + erf(-WF / np.sqrt(2.0)))
    I_diff = (Ib - Ia) * erf(WF / np.sqrt(2.0))
    tx = ku * I_diff + I_min
    ei = erfinv(2.0 * (tx - Ia) / (Ib - Ia) - 1.0)
    G = (Ib - Ia) / np.sqrt(2.0 * np.pi * sigma_b ** 2) * np.exp(-ei ** 2)
    lptx = -np.log(2.0 * WF * (Ib - Ia)) + 0.5 * LOG2PI + ei ** 2
    B = -0.5 * tx ** 2 / sn2 - np.log(G) - G ** 2 / sn2 + lptx
    C0 = (-np.log(sigma_n) - 0.5 * LOG2PI
          + np.log(2.0) - 2.0 * np.log(sigma_n)
          + 0.5 * np.log(2.0 / np.pi) - np.log(2.0)
          - 0.5 * np.log(2.0) + np.log(sigma_n))
    Bp = B + np.log(I_diff) - np.log(N_MC) + logw + C0
    order = np.argsort(G)
    return tx[order] / sn2, 2.0 * G[order] / sn2, Bp[order], G[order]


def _pack_rows(txp, g1, Bp, nk):
    """bf16 hi/lo rhs rows [8, nk] matching lhsT rows
    (xh, xh, xl, yh, yh, yl, m, m)."""
    th, tl = _split(txp[:nk])
    gh, gl = _split(g1[:nk])
    bh, bl = _split(Bp[:nk])
    return np.stack([th, tl, _split(txp[:nk])[0],
                     gh, gl, gh,
                     bh, bl]).astype(BF16)


def _build_bass(sigma_n, I1, I2, I3, logw, nk):
    """Builds the SPMD kernel graph. Scalars are compile-time constants."""
    nc = bacc.Bacc("TRN2", target_bir_lowering=False, debug=False,
                   num_devices=N_CORES)
    dt = mybir.dt.float32
    bf = mybir.dt.bfloat16
    f = mybir.ActivationFunctionType
    alu = mybir.AluOpType

    x_d = nc.dram_tensor("x", [M_PAD], dt, kind="ExternalInput").ap()
    y_d = nc.dram_tensor("y", [M_PAD], dt, kind="ExternalInput").ap()
    mask_d = nc.dram_tensor("mask", [M_PAD], dt, kind="ExternalInput").ap()
    lt8_d = nc.dram_tensor("lt8", [8, T, P], bf, kind="ExternalInput").ap()
    rhs1_d = nc.dram_tensor("rhs1", [3, 8, N_MC], bf, kind="ExternalInput").ap()
    rhs2_d = nc.dram_tensor("rhs2", [3, 8, nk], bf, kind="ExternalInput").ap()
    out_d = nc.dram_tensor("out", [1], dt, kind="ExternalOutput").ap()

    sn = sigma_n
    ck = (math.log(2.0) - math.lgamma(1.5) - 4.0 * math.log(sn)
          - 0.5 * LOG2PI)
    n2m = (nk + 511) // 512  # matmuls for the R2 region

    with tile.TileContext(nc) as tc:
        with (
            tc.tile_pool(name="singles", bufs=1) as singles,
            tc.tile_pool(name="work", bufs=2) as work,
            tc.tile_pool(name="psumA", bufs=2, space="PSUM") as psumA,
            tc.tile_pool(name="psumB", bufs=2, space="PSUM") as psumB,
            tc.tile_pool(name="dump", bufs=3) as dump,
        ):
            # ---- load inputs ----
            xs = singles.tile([P, T], dt, tag="xs")
            ys = singles.tile([P, T], dt, tag="ys")
            msk = singles.tile([P, T], dt, tag="msk")
            nc.sync.dma_start(xs[:], x_d.rearrange("(p t) -> p t", p=P))
            nc.sync.dma_start(ys[:], y_d.rearrange("(p t) -> p t", p=P))
            nc.sync.dma_start(msk[:], mask_d.rearrange("(p t) -> p t", p=P))
            lt8 = singles.tile([8, T, P], bf, tag="lt8")
            nc.sync.dma_start(lt8[:], lt8_d[:])
            rhs1 = singles.tile([8, 3, N_MC], bf, tag="rhs1")
            nc.sync.dma_start(rhs1[:], rhs1_d.rearrange("j k n -> k j n"))
            rhs2 = singles.tile([8, 3, nk], bf, tag="rhs2")
            nc.sync.dma_start(rhs2[:], rhs2_d.rearrange("j k n -> k j n"))

            # ---- per-m planes ----
            # sx2h = x^2/(2 sn^2); y2s = (y/sn)^2; lny = ln y
            sx2h = singles.tile([P, T], dt, tag="sx2h")
            y2s = singles.tile([P, T], dt, tag="y2s")
            lny = singles.tile([P, T], dt, tag="lny")
            nc.scalar.activation(sx2h[:], xs[:], f.Square,
                                 scale=1.0 / (sn * math.sqrt(2.0)))
            nc.scalar.activation(y2s[:], ys[:], f.Square, scale=1.0 / sn)
            nc.scalar.activation(lny[:], ys[:], f.Ln)
            # A = lny - sx2h - y2s
            A = singles.tile([P, T], dt, tag="A")
            tmpA = work.tile([P, T], dt, tag="tmpA")
            nc.vector.scalar_tensor_tensor(tmpA[:], sx2h[:], 1.0, y2s[:],
                                           alu.mult, alu.add)
            nc.vector.scalar_tensor_tensor(A[:], tmpA[:], -1.0, lny[:],
                                           alu.mult, alu.add)

            # interior planes -> PL[:, :, 0..2]
            PL = singles.tile([P, T, 6], dt, tag="PL")
            base = singles.tile([P, T], dt, tag="base")
            nc.vector.scalar_tensor_tensor(base[:], lny[:], 2.0, y2s[:],
                                           alu.mult, alu.subtract)
            for k, I in enumerate((I1, I2, I3)):
                qb = work.tile([P, 1], dt, tag="qb")
                nc.vector.memset(qb[:], -I / sn)
                q = work.tile([P, T], dt, tag="q")
                nc.scalar.activation(q[:], xs[:], f.Square,
                                     scale=1.0 / sn, bias=qb[:])
                basek = work.tile([P, T], dt, tag="basek")
                nc.vector.tensor_scalar_add(basek[:], base[:],
                                            ck + float(logw[k]))
                nc.vector.scalar_tensor_tensor(PL[:, :, k], q[:], -0.5,
                                               basek[:], alu.mult, alu.add)

            # ---- hot loop ----
            NM = [singles.tile([P, T], dt, tag=f"NM{j}", name=f"NM{j}")
                  for j in range(3)]
            S1 = [singles.tile([P, T], dt, tag=f"S1{j}", name=f"S1{j}")
                  for j in range(3)]
            S2 = [singles.tile([P, T], dt, tag=f"S2{j}", name=f"S2{j}")
                  for j in range(3)]
            for t in range(T):
                lhsT = lt8[:, t, :]
                for j in range(3):
                    r1 = psumA.tile([P, N_MC], dt, tag="ra")
                    for h in range(2):
                        sl = slice(h * 512, h * 512 + 512)
                        nc.tensor.matmul(r1[:, sl], lhsT, rhs1[:, j, sl][:],
                                         start=True, stop=True)
                    r2 = psumB.tile([P, nk], dt, tag="rb")
                    for h in range(n2m):
                        sl = slice(h * 512, min(h * 512 + 512, nk))
                        nc.tensor.matmul(r2[:, sl], lhsT, rhs2[:, j, sl],
                                         start=True, stop=True)
                    # coarse (negated) upper bound: max over stride-8 slice
                    sub = r1[:].rearrange("p (a b) -> p a b", b=8)[:, :, 0]
                    nc.vector.tensor_reduce(NM[j][:, t: t + 1], sub,
                                            mybir.AxisListType.X, alu.max,
                                            negate=True)
                    e1 = dump.tile([P, N_MC], dt, tag="e")
                    nc.scalar.activation(e1[:], r1[:], f.Exp,
                                         bias=NM[j][:, t: t + 1],
                                         accum_out=S1[j][:, t: t + 1])
                    e2 = dump.tile([P, nk], dt, tag="e2")
                    nc.scalar.activation(e2[:], r2[:], f.Exp,
                                         bias=NM[j][:, t: t + 1],
                                         accum_out=S2[j][:, t: t + 1])

            # ---- finalize interface planes: PL[3+j] = A - NM + ln(S1-S2) ----
            for j in range(3):
                sd = work.tile([P, T], dt, tag="sd")
                nc.vector.scalar_tensor_tensor(sd[:], S2[j][:], -1.0, S1[j][:],
                                               alu.mult, alu.add)
                lnsd = work.tile([P, T], dt, tag="lnsd")
                nc.scalar.activation(lnsd[:], sd[:], f.Ln)
                v = work.tile([P, T], dt, tag="v")
                nc.vector.scalar_tensor_tensor(v[:], NM[j][:], -1.0,
                                               lnsd[:], alu.mult, alu.add)
                nc.vector.scalar_tensor_tensor(PL[:, :, 3 + j], v[:], 1.0,
                                               A[:], alu.mult, alu.add)

            # ---- final mix: logsumexp over the 6 planes ----
            mx6 = singles.tile([P, T, 1], dt, tag="mx6")
            nc.vector.tensor_reduce(mx6[:], PL[:], mybir.AxisListType.X,
                                    alu.max)
            D = singles.tile([P, T, 6], dt, tag="D")
            nc.vector.tensor_tensor(D[:], PL[:],
                                    mx6[:].broadcast_to([P, T, 6]),
                                    alu.subtract)
            E = singles.tile([P, T, 6], dt, tag="E")
            nc.scalar.activation(E[:], D[:], f.Exp)
            sm = singles.tile([P, T, 1], dt, tag="sm")
            nc.vector.tensor_reduce(sm[:], E[:], mybir.AxisListType.X, alu.add)
            lnm = singles.tile([P, T], dt, tag="lnm")
            nc.scalar.activation(lnm[:], sm[:, :, 0], f.Ln)
            logmix = singles.tile([P, T], dt, tag="logmix")
            nc.vector.tensor_tensor(logmix[:], lnm[:], mx6[:, :, 0], alu.add)

            # ---- masked sum over all m; negate on host ----
            colsum = singles.tile([P, 1], dt, tag="colsum")
            dmp = work.tile([P, T], dt, tag="dmp")
            nc.vector.scalar_tensor_tensor(dmp[:], logmix[:], 1.0, msk[:],
                                           alu.mult, alu.mult,
                                           accum_out=colsum[:])
            total = singles.tile([P, 1], dt, tag="total")
            nc.gpsimd.partition_all_reduce(total[:], colsum[:], channels=P,
                                           reduce_op=bass_isa.ReduceOp.add)
            nc.sync.dma_start(out_d.rearrange("(p o) -> p o", p=1),
                              total[0:1, 0:1])

    nc.compile()
    return nc


def _prepare(x, y, ku12, ku23, ku13, sigma_b, sigma_n, I1, I2, I3, w):
    x = np.asarray(x, np.float32)
    y = np.asarray(y, np.float32)
    sigma_b = float(sigma_b)
    sigma_n = float(sigma_n)
    I1, I2, I3 = float(I1), float(I2), float(I3)
    w64 = np.asarray(w, np.float64)
    logw = w64 - (np.log(np.sum(np.exp(w64 - w64.max()))) + w64.max())

    # numeric-safety guard for the coarse max bound (see module docstring)
    for Ia, Ib in ((I1, I2), (I2, I3), (I1, I3)):
        L = abs(Ib - Ia) * erf(WF / np.sqrt(2.0))
        assert L * L / (2.0 * sigma_n ** 2) < 80.0, "coarse-max bound unsafe"

    y_min = float(y.min())
    g_thresh = W_SKIP * sigma_n ** 2 / (4.0 * max(y_min, 1e-6))

    terms = []
    nk = 64
    for j, (ku, Ia, Ib) in enumerate(((ku12, I1, I2), (ku23, I2, I3),
                                      (ku13, I1, I3))):
        txp, g1, Bp, G = _host_rows(np.asarray(ku), Ia, Ib, sigma_b, sigma_n,
                                    float(logw[3 + j]))
        keep = int(np.searchsorted(G, g_thresh))
        terms.append((txp, g1, Bp))
        nk = max(nk, keep)
    nk = min(N_MC, (nk + 63) // 64 * 64)

    rows1 = np.empty((3, 8, N_MC), BF16)
    rows2 = np.empty((3, 8, nk), BF16)
    for j, (txp, g1, Bp) in enumerate(terms):
        rows1[j] = _pack_rows(txp, g1, Bp, N_MC)
        r2 = _pack_rows(txp, g1, Bp, nk)
        r2[3:6] = -r2[3:6]          # negate the y*g1 rows
        rows2[j] = r2

    # lhsT rows (xh, xh, xl, yh, yh, yl, m, m) in [8, T, P] layout
    mask = np.zeros(M_PAD, np.float32)
    mask[:M_CORE] = 1.0
    mgrid = mask.reshape(P, T).T.astype(BF16)          # [T, P]

    key = (sigma_n, I1, I2, I3, tuple(np.round(logw, 12)), nk)
    if key not in _graph_cache:
        _graph_cache[key] = _build_bass(sigma_n, I1, I2, I3, logw, nk)
    nc = _graph_cache[key]

    in_maps = []
    for i in range(N_CORES):
        xi = np.full(M_PAD, 0.5, np.float32)
        yi = np.full(M_PAD, 0.5, np.float32)
        xi[:M_CORE] = x[i * M_CORE: (i + 1) * M_CORE]
        yi[:M_CORE] = y[i * M_CORE: (i + 1) * M_CORE]
        xh, xl = _split(xi.astype(np.float64))
        yh, yl = _split(yi.astype(np.float64))
        lt8 = np.empty((8, T, P), BF16)
        for r, plane in enumerate((xh, xh, xl, yh, yh, yl)):
            lt8[r] = plane.reshape(P, T).T
        lt8[6] = mgrid
        lt8[7] = mgrid
        in_maps.append({"x": xi, "y": yi, "mask": mask,
                        "lt8": lt8, "rhs1": rows1, "rhs2": rows2})
    return nc, in_maps


def kernel(x, y, ku12, ku23, ku13, sigma_b, sigma_n, I1, I2, I3, w):
    nc, in_maps = _prepare(x, y, ku12, ku23, ku13, sigma_b, sigma_n,
                           I1, I2, I3, w)
    res = run_bass_kernel_spmd(nc, in_maps, core_ids=list(range(N_CORES)))
    global _last_results
    _last_results = res
    partials = [float(res.results[i]["out"][0]) for i in range(N_CORES)]
    return np.float32(-np.sum(partials))


# revision 15
# speedup vs baseline: 8.3234x; 1.8370x over previous
"""Trainium2 Bass kernel for the ArcModel3Phase loss.

Math restructuring (vs the reference):
  Each MC interface term needs logsumexp_n(lpx + lpy + lptx) over N=1024
  samples for each of M points.  Expanding all three log-densities,
      l_nm = A_m + B_n + x_m*(tx_n/sn^2) + y_m*(2 G_n/sn^2)
             + log(1 - exp(-(4/sn^2) y_m G_n))
  The affine part R1_nm = x_m*txp_n + y_m*g1_n + B_n is a matmul, and with
  w_nm = (4/sn^2) y G = R1 - R2 where R2_nm = x_m*txp_n - y_m*g1_n + B_n
  (same matmul, g1 negated),
      sum_n e^{l - b} = sum_n e^{R1-b} - sum_n e^{R2-b}
  for any bound b -- A_m cancels, and b only affects numerics: a drop of
  up to (Ib-Ia)^2/(2 sn^2) ~ 72 below the true max keeps every f32 value
  finite.  w >= 0.2 here, so the subtraction loses < 3 bits.

Three accuracy-preserving device optimizations:
  1. Sample pairing: tx-adjacent samples merge, e^ha + e^hb =
     2 cosh(d) e^{(ha+hb)/2} with d = (ha-hb)/2.  ln cosh(d) ~ d^2/2 is
     carried EXACTLY to second order as extra matmul rows, since d is
     affine in (x, y): d^2/2 contributes x^2, y^2, xy, x, y, 1 terms.
     N halves to 512 per term; residual error is O(d^4).
  2. The mixture only needs SUM_j e^{plane_j}, so one shared bound b per
     m lets all three terms accumulate in a single fused exp+accum pass
     over a concatenated [128, 3*512] PSUM region (2 ScalarE passes and
     2 accumulator drains per tile instead of 6+6).
  3. Merged samples sorted by G; those with w >= W_SKIP for every m
     (G >= W_SKIP*sn^2/(4 y_min)) contribute < e^-W_SKIP relative to s2
     and are skipped in the R2/e2 pass.

fp32 matmul streams at 1/4 PE rate, so factors are split hi/lo into bf16
(x*t = xh*th + xh*tl + xl*th, ~2^-17 relative; correction rows single
bf16).  The K=14 bf16 matmul streams at full rate.

Per-core layout: M=100000 sharded 8 ways -> 12500, padded to 12544 =
128 partitions x 98 tiles (m = p*98 + t), with a 0/1 mask for the pad.
The mask doubles as the "ones" lhsT row (pad garbage is masked out).
"""
import math

import numpy as np
import ml_dtypes
from scipy.special import erf, erfinv

import concourse.bass as bass
import concourse.tile as tile
from concourse import bacc, mybir
from concourse import bass_isa
from concourse.bass_utils import run_bass_kernel_spmd

WF = 3.0
LOG2PI = math.log(2.0 * math.pi)
M = 100_000
N_MC = 1024
NP = N_MC // 2                 # merged samples per term
N_CORES = 8
M_CORE = M // N_CORES          # 12500
P = 128
T = 98                         # tiles per core; P*T = 12544 >= M_CORE
M_PAD = P * T
W_SKIP = 9.0                   # skip e2 samples with w >= this for all m
K_ROWS = 14
BF16 = ml_dtypes.bfloat16

_graph_cache = {}
_last_results = None


def _split(a):
    hi = a.astype(BF16)
    lo = (a - hi.astype(np.float64)).astype(BF16)
    return hi, lo


def _host_rows(ku, Ia, Ib, sigma_b, sigma_n, logw):
    """Merged per-sample rows for one interface term (float64 math).

    Returns (tm, gm, Bm, dt, dg, dB, Gm): pair means, half-differences,
    and the pair min G, sorted by Gm ascending.
    """
    ku = ku.astype(np.float64)
    sn2 = sigma_n ** 2
    I_min = Ia + 0.5 * (Ib - Ia) * (1.0 + erf(-WF / np.sqrt(2.0)))
    I_diff = (Ib - Ia) * erf(WF / np.sqrt(2.0))
    tx = np.sort(ku * I_diff + I_min)
    ei = erfinv(2.0 * (tx - Ia) / (Ib - Ia) - 1.0)
    G = (Ib - Ia) / np.sqrt(2.0 * np.pi * sigma_b ** 2) * np.exp(-ei ** 2)
    lptx = -np.log(2.0 * WF * (Ib - Ia)) + 0.5 * LOG2PI + ei ** 2
    B = -0.5 * tx ** 2 / sn2 - np.log(G) - G ** 2 / sn2 + lptx
    C0 = (-np.log(sigma_n) - 0.5 * LOG2PI
          + np.log(2.0) - 2.0 * np.log(sigma_n)
          + 0.5 * np.log(2.0 / np.pi) - np.log(2.0)
          - 0.5 * np.log(2.0) + np.log(sigma_n))
    Bp = B + np.log(I_diff) - np.log(N_MC) + logw + C0
    txp, g1 = tx / sn2, 2.0 * G / sn2

    tm = 0.5 * (txp[0::2] + txp[1::2])
    gm = 0.5 * (g1[0::2] + g1[1::2])
    Bm = 0.5 * (Bp[0::2] + Bp[1::2]) + np.log(2.0)
    dt = 0.5 * (txp[0::2] - txp[1::2])
    dg = 0.5 * (g1[0::2] - g1[1::2])
    dB = 0.5 * (Bp[0::2] - Bp[1::2])
    Gm = np.minimum(G[0::2], G[1::2])
    o = np.argsort(Gm)
    return tm[o], gm[o], Bm[o], dt[o], dg[o], dB[o], Gm[o]


def _pack_rows(tm, gm, Bm, dt, dg, dB, sign, sl):
    """bf16 rhs rows [14, n] for one region.  sign=+1 for R1, -1 for R2.
    lhsT rows: (xh, xh, xl, yh, yh, yl, m, m, x2, y2, xy, xh, yh, m)."""
    th, tl = _split(tm[sl])
    gh, gl = _split(sign * gm[sl])
    bh, bl = _split(Bm[sl] + 0.5 * dB[sl] ** 2)
    return np.stack([
        th, tl, th,
        gh, gl, gh,
        bh, bl,
        (0.5 * dt[sl] ** 2).astype(BF16),
        (0.5 * dg[sl] ** 2).astype(BF16),
        (sign * dt[sl] * dg[sl]).astype(BF16),
        (dt[sl] * dB[sl]).astype(BF16),
        (sign * dg[sl] * dB[sl]).astype(BF16),
        np.zeros(len(dB[sl]), BF16),       # dB^2/2 folded into B rows
    ]).astype(BF16)


def _build_bass(sigma_n, I1, I2, I3, logw, nks):
    """Builds the SPMD kernel graph. Scalars are compile-time constants."""
    nc = bacc.Bacc("TRN2", target_bir_lowering=False, debug=False,
                   num_devices=N_CORES)
    dt_ = mybir.dt.float32
    bf = mybir.dt.bfloat16
    f = mybir.ActivationFunctionType
    alu = mybir.AluOpType

    N1T = 3 * NP                       # combined R1 columns
    nkt = sum(nks)                     # combined R2 columns

    x_d = nc.dram_tensor("x", [M_PAD], dt_, kind="ExternalInput").ap()
    y_d = nc.dram_tensor("y", [M_PAD], dt_, kind="ExternalInput").ap()
    mask_d = nc.dram_tensor("mask", [M_PAD], dt_, kind="ExternalInput").ap()
    lt_d = nc.dram_tensor("lt", [K_ROWS, T, P], bf, kind="ExternalInput").ap()
    rhs1_d = nc.dram_tensor("rhs1", [K_ROWS, N1T], bf,
                            kind="ExternalInput").ap()
    rhs2_d = nc.dram_tensor("rhs2", [K_ROWS, nkt], bf,
                            kind="ExternalInput").ap()
    out_d = nc.dram_tensor("out", [1], dt_, kind="ExternalOutput").ap()

    sn = sigma_n
    ck = (math.log(2.0) - math.lgamma(1.5) - 4.0 * math.log(sn)
          - 0.5 * LOG2PI)

    with tile.TileContext(nc) as tc:
        with (
            tc.tile_pool(name="singles", bufs=1) as singles,
            tc.tile_pool(name="work", bufs=2) as work,
            tc.tile_pool(name="psumA", bufs=2, space="PSUM") as psumA,
            tc.tile_pool(name="psumB", bufs=2, space="PSUM") as psumB,
            tc.tile_pool(name="dump", bufs=3) as dump,
        ):
            # ---- load inputs ----
            xs = singles.tile([P, T], dt_, tag="xs")
            ys = singles.tile([P, T], dt_, tag="ys")
            msk = singles.tile([P, T], dt_, tag="msk")
            nc.sync.dma_start(xs[:], x_d.rearrange("(p t) -> p t", p=P))
            nc.sync.dma_start(ys[:], y_d.rearrange("(p t) -> p t", p=P))
            nc.sync.dma_start(msk[:], mask_d.rearrange("(p t) -> p t", p=P))
            lt = singles.tile([K_ROWS, T, P], bf, tag="lt")
            nc.sync.dma_start(lt[:], lt_d[:])
            rhs1 = singles.tile([K_ROWS, N1T], bf, tag="rhs1")
            nc.sync.dma_start(rhs1[:], rhs1_d[:])
            rhs2 = singles.tile([K_ROWS, nkt], bf, tag="rhs2")
            nc.sync.dma_start(rhs2[:], rhs2_d[:])

            # ---- per-m planes ----
            sx2h = singles.tile([P, T], dt_, tag="sx2h")
            y2s = singles.tile([P, T], dt_, tag="y2s")
            lny = singles.tile([P, T], dt_, tag="lny")
            nc.scalar.activation(sx2h[:], xs[:], f.Square,
                                 scale=1.0 / (sn * math.sqrt(2.0)))
            nc.scalar.activation(y2s[:], ys[:], f.Square, scale=1.0 / sn)
            nc.scalar.activation(lny[:], ys[:], f.Ln)
            # A = lny - sx2h - y2s
            A = singles.tile([P, T], dt_, tag="A")
            tmpA = work.tile([P, T], dt_, tag="tmpA")
            nc.vector.scalar_tensor_tensor(tmpA[:], sx2h[:], 1.0, y2s[:],
                                           alu.mult, alu.add)
            nc.vector.scalar_tensor_tensor(A[:], tmpA[:], -1.0, lny[:],
                                           alu.mult, alu.add)

            # interior planes -> PL[:, :, 0..2]
            PL = singles.tile([P, T, 4], dt_, tag="PL")
            base = singles.tile([P, T], dt_, tag="base")
            nc.vector.scalar_tensor_tensor(base[:], lny[:], 2.0, y2s[:],
                                           alu.mult, alu.subtract)
            for k, I in enumerate((I1, I2, I3)):
                qb = work.tile([P, 1], dt_, tag="qb")
                nc.vector.memset(qb[:], -I / sn)
                q = work.tile([P, T], dt_, tag="q")
                nc.scalar.activation(q[:], xs[:], f.Square,
                                     scale=1.0 / sn, bias=qb[:])
                basek = work.tile([P, T], dt_, tag="basek")
                nc.vector.tensor_scalar_add(basek[:], base[:],
                                            ck + float(logw[k]))
                nc.vector.scalar_tensor_tensor(PL[:, :, k], q[:], -0.5,
                                               basek[:], alu.mult, alu.add)

            # ---- hot loop: one fused interface pass per tile ----
            NM = singles.tile([P, T], dt_, tag="NM")
            S1 = singles.tile([P, T], dt_, tag="S1")
            S2 = singles.tile([P, T], dt_, tag="S2")
            k2o = np.concatenate([[0], np.cumsum(nks)]).tolist()
            for t in range(T):
                lhsT = lt[:, t, :]
                r1 = psumA.tile([P, N1T], dt_, tag="ra")
                for j in range(3):
                    sl = slice(j * NP, (j + 1) * NP)
                    nc.tensor.matmul(r1[:, sl], lhsT, rhs1[:, sl],
                                     start=True, stop=True)
                r2 = psumB.tile([P, nkt], dt_, tag="rb")
                for j in range(3):
                    sl = slice(k2o[j], k2o[j + 1])
                    nc.tensor.matmul(r2[:, sl], lhsT, rhs2[:, sl],
                                     start=True, stop=True)
                # coarse (negated) shared upper bound over stride-8 slice
                sub = r1[:].rearrange("p (a b) -> p a b", b=8)[:, :, 0]
                nc.vector.tensor_reduce(NM[:, t: t + 1], sub,
                                        mybir.AxisListType.X, alu.max,
                                        negate=True)
                e1 = dump.tile([P, N1T], dt_, tag="e")
                nc.scalar.activation(e1[:], r1[:], f.Exp,
                                     bias=NM[:, t: t + 1],
                                     accum_out=S1[:, t: t + 1])
                e2 = dump.tile([P, nkt], dt_, tag="e2")
                nc.scalar.activation(e2[:], r2[:], f.Exp,
                                     bias=NM[:, t: t + 1],
                                     accum_out=S2[:, t: t + 1])

            # ---- finalize interface plane: PL[3] = A - NM + ln(S1-S2) ----
            sd = work.tile([P, T], dt_, tag="sd")
            nc.vector.scalar_tensor_tensor(sd[:], S2[:], -1.0, S1[:],
                                           alu.mult, alu.add)
            lnsd = work.tile([P, T], dt_, tag="lnsd")
            nc.scalar.activation(lnsd[:], sd[:], f.Ln)
            v = work.tile([P, T], dt_, tag="v")
            nc.vector.scalar_tensor_tensor(v[:], NM[:], -1.0,
                                           lnsd[:], alu.mult, alu.add)
            nc.vector.scalar_tensor_tensor(PL[:, :, 3], v[:], 1.0,
                                           A[:], alu.mult, alu.add)

            # ---- final mix: logsumexp over the 4 planes ----
            mx6 = singles.tile([P, T, 1], dt_, tag="mx6")
            nc.vector.tensor_reduce(mx6[:], PL[:], mybir.AxisListType.X,
                                    alu.max)
            D = singles.tile([P, T, 4], dt_, tag="D")
            nc.vector.tensor_tensor(D[:], PL[:],
                                    mx6[:].broadcast_to([P, T, 4]),
                                    alu.subtract)
            E = singles.tile([P, T, 4], dt_, tag="E")
            nc.scalar.activation(E[:], D[:], f.Exp)
            sm = singles.tile([P, T, 1], dt_, tag="sm")
            nc.vector.tensor_reduce(sm[:], E[:], mybir.AxisListType.X,
                                    alu.add)
            lnm = singles.tile([P, T], dt_, tag="lnm")
            nc.scalar.activation(lnm[:], sm[:, :, 0], f.Ln)
            logmix = singles.tile([P, T], dt_, tag="logmix")
            nc.vector.tensor_tensor(logmix[:], lnm[:], mx6[:, :, 0], alu.add)

            # ---- masked sum over all m; negate on host ----
            colsum = singles.tile([P, 1], dt_, tag="colsum")
            dmp = work.tile([P, T], dt_, tag="dmp")
            nc.vector.scalar_tensor_tensor(dmp[:], logmix[:], 1.0, msk[:],
                                           alu.mult, alu.mult,
                                           accum_out=colsum[:])
            total = singles.tile([P, 1], dt_, tag="total")
            nc.gpsimd.partition_all_reduce(total[:], colsum[:], channels=P,
                                           reduce_op=bass_isa.ReduceOp.add)
            nc.sync.dma_start(out_d.rearrange("(p o) -> p o", p=1),
                              total[0:1, 0:1])

    nc.compile()
    return nc


def _prepare(x, y, ku12, ku23, ku13, sigma_b, sigma_n, I1, I2, I3, w):
    x = np.asarray(x, np.float32)
    y = np.asarray(y, np.float32)
    sigma_b = float(sigma_b)
    sigma_n = float(sigma_n)
    I1, I2, I3 = float(I1), float(I2), float(I3)
    w64 = np.asarray(w, np.float64)
    logw = w64 - (np.log(np.sum(np.exp(w64 - w64.max()))) + w64.max())

    # numeric-safety guard for the coarse shared max bound
    for Ia, Ib in ((I1, I2), (I2, I3), (I1, I3)):
        L = abs(Ib - Ia) * erf(WF / np.sqrt(2.0))
        assert L * L / (2.0 * sigma_n ** 2) < 80.0, "coarse-max bound unsafe"

    y_min = float(y.min())
    g_thresh = W_SKIP * sigma_n ** 2 / (4.0 * max(y_min, 1e-6))

    terms = []
    nks = []
    for j, (ku, Ia, Ib) in enumerate(((ku12, I1, I2), (ku23, I2, I3),
                                      (ku13, I1, I3))):
        tr = _host_rows(np.asarray(ku), Ia, Ib, sigma_b, sigma_n,
                        float(logw[3 + j]))
        keep = int(np.searchsorted(tr[6], g_thresh))
        nk = min(NP, (max(keep, 32) + 31) // 32 * 32)
        terms.append(tr)
        nks.append(nk)
    if sum(nks) > 512:
        # each R2 matmul output slice must stay inside one PSUM bank
        nks = [NP, NP, NP]

    rows1 = np.concatenate(
        [_pack_rows(*tr[:6], +1.0, slice(None)) for tr in terms], axis=1)
    rows2 = np.concatenate(
        [_pack_rows(*tr[:6], -1.0, slice(0, nk))
         for tr, nk in zip(terms, nks)], axis=1)

    # lhsT rows (xh,xh,xl, yh,yh,yl, m,m, x2,y2,xy, xh,yh,m) in [14,T,P]
    mask = np.zeros(M_PAD, np.float32)
    mask[:M_CORE] = 1.0
    mgrid = mask.reshape(P, T).T.astype(BF16)          # [T, P]

    key = (sigma_n, I1, I2, I3, tuple(np.round(logw, 12)), tuple(nks))
    if key not in _graph_cache:
        _graph_cache[key] = _build_bass(sigma_n, I1, I2, I3, logw, nks)
    nc = _graph_cache[key]

    in_maps = []
    for i in range(N_CORES):
        xi = np.full(M_PAD, 0.5, np.float64)
        yi = np.full(M_PAD, 0.5, np.float64)
        xi[:M_CORE] = x[i * M_CORE: (i + 1) * M_CORE]
        yi[:M_CORE] = y[i * M_CORE: (i + 1) * M_CORE]
        xh, xl = _split(xi)
        yh, yl = _split(yi)
        x2 = (xi * xi).astype(BF16)
        y2 = (yi * yi).astype(BF16)
        xy = (xi * yi).astype(BF16)
        lt = np.empty((K_ROWS, T, P), BF16)
        planes = (xh, xh, xl, yh, yh, yl, None, None, x2, y2, xy, xh, yh,
                  None)
        for r, plane in enumerate(planes):
            lt[r] = mgrid if plane is None else plane.reshape(P, T).T
        in_maps.append({"x": xi.astype(np.float32),
                        "y": yi.astype(np.float32), "mask": mask,
                        "lt": lt, "rhs1": rows1, "rhs2": rows2})
    return nc, in_maps


def kernel(x, y, ku12, ku23, ku13, sigma_b, sigma_n, I1, I2, I3, w):
    nc, in_maps = _prepare(x, y, ku12, ku23, ku13, sigma_b, sigma_n,
                           I1, I2, I3, w)
    res = run_bass_kernel_spmd(nc, in_maps, core_ids=list(range(N_CORES)))
    global _last_results
    _last_results = res
    partials = [float(res.results[i]["out"][0]) for i in range(N_CORES)]
    return np.float32(-np.sum(partials))
